# revision 3
# baseline (speedup 1.0000x reference)
"""BernsteinConv Trainium2 kernel (dev version: imports kernel_lib/bassrun)."""
import sys
import numpy as np

sys.path.insert(0, "/root/problem")
from bassrun import new_nc, run, mybir, tile, bass  # noqa: E402
import kernel_lib as KL  # noqa: E402

NC = 8
BG = 32
_cache = {}


def kernel(feat, edge_src, edge_dst):
    feat = np.asarray(feat, dtype=np.float32)
    edge_src = np.asarray(edge_src)
    edge_dst = np.asarray(edge_dst)
    N = feat.shape[0]
    NPC = (N + NC - 1) // NC

    in_maps, meta = KL.preprocess(feat, edge_src, edge_dst, NC, NPC, BW=5,
                                  BG=BG)
    BW = 5
    if meta["maxcnt"] > BW * 128:
        BW = (meta["maxcnt"] + 127) // 128
        in_maps, meta = KL.preprocess(feat, edge_src, edge_dst, NC, NPC,
                                      BW=BW, BG=BG)

    key = (N, meta["NBLK"], meta["idx_cols"])
    if key not in _cache:
        nc = new_nc(num_devices=NC, num_swdge_queues=4,
                    scratch=int(__import__("os").environ.get("KSCRATCH", "16384")))
        KL.build(nc, tile, mybir, bass, meta, BW, BG, NQ=4)
        nc.compile()
        _cache[key] = nc
    nc = _cache[key]

    trace = bool(getattr(kernel, "trace", False))
    res = run(nc, in_maps, n_cores=NC, trace=trace, compile=False)
    kernel.last_exec_time_ns = res.exec_time_ns
    out = KL.postprocess(res.results, N, NPC, meta["NWIN"])
    return out
